# revision 55
# baseline (speedup 1.0000x reference)
"""Bass/Trainium2 kernel for nn_BatchLoreAttentionLayer.

Reference math (per batch item b, X = embeddings[b] in [L=128, D=256]):
    Q = X q_w^T + q_b ; K = X k_w^T + k_b
    S = Q K^T / sqrt(D) ; S[:, padded] = -inf
    attn = softmax_m(S) ; attended = attn X
    out = tanh( (valid^T attended) / cnt )

Algebraic restructure (same as v1):
    S = X A X^T / sqrt(D) + row_const(l) + s(m),  A = q_w^T k_w
    out_b = tanh( w X ),  w[m] = sum_l g[l] E[l,m],
    E = exp(S + rowbias),  g[l] = valid[b,l] / (rowsum[l] * cnt_b)

v2: fp8 score path with DoubleRow matmuls (89.4us cost model vs 120.9 v1;
    measured end-to-end rel err 1.692e-2 against tol 2e-2, deterministic).
    A and X^T ship as fp8e4m3 in [128, 2, .] two-k-tile layout; Yt = A^T Xt and
    S = Yt^T Xt run as fp8 DoubleRow matmuls (256-contraction, 0.5 cyc/row).
    A is pre-scaled by 256 on host; exp() descales via ACT scale=1/256.
    Mask bias enters S as rank-1 DR matmuls (ones=128 x rb=-160 -> -20480);
    exactly one start=True per touched 2KB PSUM bank and one stop=True on the
    bank's last matmul -- a second start re-marks the whole bank pending-zero
    and later accumulates drop earlier writes (real-HW semantics, matches
    CoreSim's lazy zero-region model).
    Values (xl) ship as fp8e3m4 [l, d] (4 mantissa bits, |X| < 15.5); out bf16.

    Engine split per 8-item group (GPSIMD cannot touch PSUM or reduce along
    the free axis): PE all matmuls; ACT exp + 2 Yt-granule copies (the wall,
    ~2.33us/slot, 100% busy in steady state); DVE 2 Yt copies + short rowsum
    reduce + recip + w copy; GPSIMD two fold-adds (rowsum tree) + g-mul;
    DMA 17.3MB/core at 360B/ns. 8 PSUM banks: 3-buf Yt ring (4 granules +
    w tile) | 2x 2-bank S | 1 out. Slot-pipelined emission with per-stage
    lags (s-1, exp-2, fold-3/4, red-5, gmul-6, w-7, o-8); per-granule SBUF
    tiles avoid cross-engine per-tile WAW serialization; all recurring DMA
    issues on the SP queue (an ACT-queue DMA issue stalls exp dispatch);
    one-time loads spread over slots 0/5/8 to stay under the 8 in-flight
    DMA lane limit.

Sharding: pure data-parallel over B across 8 cores (256 items/core).
"""

import os
import sys
from contextlib import ExitStack

import numpy as np
import ml_dtypes

sys.path.insert(0, "/opt/trn_rl_repo")

import concourse.bass as bass  # noqa: E402
import concourse.mybir as mybir  # noqa: E402
import concourse.tile as tile  # noqa: E402
from concourse import bacc  # noqa: E402
from concourse.bass import ts  # noqa: E402
from concourse.bass_utils import run_bass_kernel_spmd  # noqa: E402

B, L, D = 2048, 128, 256
NCORES = 8
BPC = B // NCORES   # items per core
CHUNK = 128         # items per output accumulation chunk
GRP = 8             # items per vector-op group
NG = BPC // GRP     # groups per core
GPC = CHUNK // GRP  # groups per chunk

F32 = mybir.dt.float32
BF16 = mybir.dt.bfloat16
FP8 = mybir.dt.float8e4    # e4m3, DoubleRow capable
FP8V = mybir.dt.float8e3   # e3m4, values
AF = mybir.ActivationFunctionType
DR = mybir.MatmulPerfMode.DoubleRow

# host-tunable knobs (affect compiled program; keep consistent per process)
VAL_BF16 = os.environ.get("KVAL", "e3") == "bf16"  # values dtype
POOL_COPIES = os.environ.get("KPOOL", "1") == "1"  # gpsimd yt copies
ASC = 256.0  # A pre-scale

_CACHE = {}


def build_bass():
    nc = bacc.Bacc(None, target_bir_lowering=False)
    xt = nc.declare_dram_parameter("xt", [NG, 128, 2 * GRP * L], FP8, isOutput=False)
    xl = nc.declare_dram_parameter(
        "xl", [NG, 128, GRP * D], BF16 if VAL_BF16 else FP8V, isOutput=False
    )
    rb = nc.declare_dram_parameter("rb", [1, NG, 2, GRP * L], FP8, isOutput=False)
    vt = nc.declare_dram_parameter("vt", [128, BPC], F32, isOutput=False)
    aw = nc.declare_dram_parameter("aw", [128, 2 * D], FP8, isOutput=False)
    outT = nc.declare_dram_parameter(
        "outT", [BPC // CHUNK, 128, 2, CHUNK], BF16, isOutput=True
    )
    build_body(nc, xt, xl, rb, vt, aw, outT)
    nc.finalize()
    return nc


def build_body(nc, xt, xl, rb, vt, aw, outT):
    with tile.TileContext(nc) as tc, ExitStack() as ctx:
        singles = ctx.enter_context(tc.tile_pool(name="singles", bufs=1))
        io_xt = ctx.enter_context(tc.tile_pool(name="io_xt", bufs=6))
        io_xl = ctx.enter_context(tc.tile_pool(name="io_xl", bufs=6))
        work = ctx.enter_context(tc.tile_pool(name="work", bufs=12))
        pool_e = ctx.enter_context(tc.tile_pool(name="pool_e", bufs=8))
        small = ctx.enter_context(tc.tile_pool(name="small", bufs=6))
        ps_yt = ctx.enter_context(tc.tile_pool(name="ps_yt", bufs=3, space="PSUM"))
        ps_s = ctx.enter_context(tc.tile_pool(name="ps_s", bufs=2, space="PSUM"))
        ps_o = ctx.enter_context(tc.tile_pool(name="ps_o", bufs=1, space="PSUM"))

        # ---- one-time loads (aw first: it gates the first yt matmul; the
        # bulk rb/vt loads go behind the first xt prefetches) ----
        a_sb = singles.tile([128, 2, D], FP8)  # [d_sub, d_hi, e], A*ASC
        nc.scalar.dma_start(out=a_sb, in_=aw.rearrange("p (t e) -> p t e", t=2))
        ones_dr = singles.tile([1, 2, 128], FP8)
        nc.vector.memset(ones_dr, 128.0)
        vt_sb = singles.tile([128, BPC], F32)  # valid/cnt, [l, b]
        rb_sb = singles.tile([1, NG, 2, GRP * L], FP8)

        def load_singles(step):
            # spread across early slots: <=8 DMAs in flight (DMAHW lanes) and
            # the ACT SEQ stays free for the first granule-copy dispatches
            half = NG // 2
            if step == 0:
                nc.scalar.dma_start(out=rb_sb[:, 0:half], in_=rb[:, 0:half])
            elif step == 5:
                nc.scalar.dma_start(out=vt_sb, in_=vt[:, :])
            elif step == 8:
                nc.scalar.dma_start(out=rb_sb[:, half:NG], in_=rb[:, half:NG])

        xt8 = [None] * NG
        xlv = [None] * NG
        yt_sb = [None] * NG
        sps = [None] * NG
        e_sb = [None] * NG
        rinv = [None] * NG
        g8 = [None] * NG
        w8 = [None] * NG
        oT_ps = [None] * (BPC // CHUNK)
        oT_sb = [None] * (BPC // CHUNK)

        def load_xt(g):
            xt8[g] = io_xt.tile([128, 2, GRP * L], FP8, tag="xt", name="xt8")
            nc.sync.dma_start(
                out=xt8[g], in_=xt.rearrange("n p (t x) -> n p t x", t=2)[g]
            )

        def load_xl(g):
            xlv[g] = io_xl.tile(
                [128, GRP * D], BF16 if VAL_BF16 else FP8V, tag="xl", name="xlv"
            )
            nc.sync.dma_start(out=xlv[g], in_=xl[g])

        def yt_mms(g):
            # Yt = (A*ASC)^T Xt in 4 granules of 2 items; copy to SBUF fp8.
            # Each granule gets its own SBUF tile so the four copies (on
            # different engines) carry no WAW ordering between them.
            yt_sb[g] = []
            for gr in range(4):
                ytp = ps_yt.tile([128, 2, 2 * L], F32, tag="yt")
                for h in range(2):
                    nc.tensor.matmul(
                        out=ytp[:, h, :],
                        lhsT=a_sb[:, :, ts(h, 128)],
                        rhs=xt8[g][:, :, ts(gr, 2 * L)],
                        start=True,
                        stop=True,
                        perf_mode=DR,
                    )
                dst = work.tile([128, 2, 2 * L], FP8, tag="ytsb", name="ytg")
                yt_sb[g].append(dst)
                if gr < 2:  # GPSIMD cannot read PSUM: copies on ACT + DVE
                    nc.scalar.activation(out=dst, in_=ytp, func=AF.Copy)
                else:
                    nc.vector.tensor_copy(out=dst, in_=ytp)

        def s_mms(g):
            # S*ASC for 8 items into one 2-bank PSUM tile + rank-1 mask bias.
            sp = ps_s.tile([128, GRP, L], F32, tag="s", name="spst")
            sps[g] = sp
            sp_flat = sp.rearrange("p g l -> p (g l)")
            # One start per 2KB PSUM bank (a second start re-marks the whole
            # bank pending-zero and the later accumulates drop prior writes);
            # one stop per bank on the last matmul touching it.
            for q in range(4):  # bias first
                nc.tensor.matmul(
                    out=sp_flat[:, ts(q, 2 * L)],
                    lhsT=ones_dr,
                    rhs=rb_sb[:, g, :, ts(q, 2 * L)],
                    start=(q % 2 == 0),
                    stop=False,
                    perf_mode=DR,
                )
            for j in range(GRP):
                nc.tensor.matmul(
                    out=sp[:, j, :],
                    lhsT=yt_sb[g][j // 2][:, :, ts(j % 2, L)],
                    rhs=xt8[g][:, :, ts(j, L)],
                    start=False,
                    stop=(j % 4 == 3),
                    perf_mode=DR,
                )

        def exp_phase(g):
            e_sb[g] = pool_e.tile([128, GRP, L], BF16, tag="e", name="esb")
            nc.scalar.activation(
                out=e_sb[g], in_=sps[g], func=AF.Exp, scale=float(1.0 / ASC)
            )

        ef1 = [None] * NG
        ef2 = [None] * NG

        def fold1_phase(g):
            # rowsum stage 1 on GPSIMD (SBUF->SBUF): E halves added, f32 out
            ef1[g] = pool_e.tile([128, GRP, L // 2], F32, tag="ef1", name="ef1t")
            nc.gpsimd.tensor_add(
                ef1[g], e_sb[g][:, :, 0 : L // 2], e_sb[g][:, :, L // 2 : L]
            )

        def fold2_phase(g):
            ef2[g] = pool_e.tile([128, GRP, L // 4], F32, tag="ef2", name="ef2t")
            nc.gpsimd.tensor_add(
                ef2[g], ef1[g][:, :, 0 : L // 4], ef1[g][:, :, L // 4 : L // 2]
            )

        def red_phase(g):
            rs = small.tile([128, GRP], F32, tag="rs")
            nc.vector.reduce_sum(out=rs, in_=ef2[g], axis=mybir.AxisListType.X)
            rinv[g] = small.tile([128, GRP], F32, tag="rinv", name="rinvt")
            nc.vector.reciprocal(out=rinv[g], in_=rs)

        def gmul_phase(g):
            g8[g] = small.tile([128, GRP], BF16, tag="g8", name="g8t")
            nc.gpsimd.tensor_mul(
                g8[g], rinv[g], vt_sb[:, g * GRP : (g + 1) * GRP]
            )

        def w_mms(g):
            # w[m] = sum_l E[l,m] g[l] per item -> PSUM, copy to SBUF bf16.
            wps = ps_yt.tile([128, GRP], F32, tag="yt", name="wpst")
            for j in range(GRP):
                nc.tensor.matmul(
                    out=wps[:, j : j + 1],
                    lhsT=e_sb[g][:, j, :],
                    rhs=g8[g][:, j : j + 1],
                    start=True,
                    stop=True,
                )
            w8[g] = small.tile([128, GRP], BF16, tag="w8", name="w8t")
            nc.vector.tensor_copy(out=w8[g], in_=wps)

        def o_mms(g):
            c = g // GPC
            if g % GPC == 0:
                oT_ps[c] = ps_o.tile([128, 2, CHUNK], F32, tag="oT", name="oTps")
            col0 = (g % GPC) * GRP
            for j in range(GRP):
                for dh in range(2):
                    nc.tensor.matmul(
                        out=oT_ps[c][:, dh, col0 + j : col0 + j + 1],
                        lhsT=xlv[g].rearrange("p (i d) -> p i d", i=GRP)[
                            :, j, ts(dh, 128)
                        ],
                        rhs=w8[g][:, j : j + 1],
                        start=True,
                        stop=True,
                    )
            if g % GPC == GPC - 1:  # chunk complete -> tanh + store
                oT_sb[c] = work.tile(
                    [128, 2, CHUNK], BF16, tag="oT_sb", name="oTsb"
                )
                nc.scalar.activation(out=oT_sb[c], in_=oT_ps[c], func=AF.Tanh)
                nc.sync.dma_start(out=outT[c], in_=oT_sb[c])

        # ---- flat slot pipeline: each engine's stream lags the chain so no
        # in-order queue waits on a dependency that isn't already satisfied.
        PF = 3
        for slot in range(NG + 9):
            if slot == 0:
                for i in range(PF + 1):
                    load_xt(i)
            if slot in (0, 5, 8):
                load_singles(slot)
            if 2 <= slot and slot - 2 < NG:
                exp_phase(slot - 2)
            if 1 <= slot and slot - 1 < NG:
                s_mms(slot - 1)
            if slot < NG:
                yt_mms(slot)
            if 3 <= slot and slot - 3 < NG:
                fold1_phase(slot - 3)
            if 4 <= slot and slot - 4 < NG:
                fold2_phase(slot - 4)
            if 5 <= slot and slot - 5 < NG:
                red_phase(slot - 5)
            if 6 <= slot and slot - 6 < NG:
                gmul_phase(slot - 6)
            if 7 <= slot and slot - 7 < NG:
                w_mms(slot - 7)
            if 8 <= slot and slot - 8 < NG:
                o_mms(slot - 8)
            if 1 <= slot and slot + PF < NG:
                load_xt(slot + PF)
            if 3 <= slot and slot - 3 < NG:
                load_xl(slot - 3)


def _get_nc():
    if "nc" not in _CACHE:
        _CACHE["nc"] = build_bass()
    return _CACHE["nc"]


def prep_inputs(embeddings, padding_mask, q_w, q_b, k_w, k_b):
    """Host-side shard prep: dtype casts, weight folding, mask/count folding."""
    emb = np.asarray(embeddings, np.float32)
    mask = np.asarray(padding_mask)
    q_w = np.asarray(q_w, np.float32)
    k_w = np.asarray(k_w, np.float32)
    q_b = np.asarray(q_b, np.float32)
    scale = 1.0 / np.sqrt(np.float32(D))

    f8 = ml_dtypes.float8_e4m3
    bf = ml_dtypes.bfloat16
    vdt = bf if VAL_BF16 else ml_dtypes.float8_e3m4

    A = (q_w.T @ k_w) * (scale * ASC)              # [D, D] (d, e), pre-scaled
    v = (k_w.T @ q_b) * scale                      # [D]
    # bias rows enter S*ASC via ones(=128) x rb -> 128*rb; exp descales by ASC
    rowbias = np.where(mask, np.float32(-160.0), np.float32(0.0))
    if np.any(v):
        rowbias = rowbias + (emb @ v) * np.float32(ASC / 128.0)
    valid = (~mask).astype(np.float32)             # [B, L]
    cnt = np.maximum(valid.sum(1, keepdims=True), 1.0)
    vt_full = (valid / cnt).T.astype(np.float32)   # [L, B]

    # aw: [d_sub(128), d_hi(2), e(256)] flattened to [128, 512]
    awq = np.ascontiguousarray(
        A.reshape(2, 128, D).transpose(1, 0, 2).reshape(128, 2 * D)
    ).astype(f8)
    # xt: [B/GRP, d_sub(128), d_hi(2), item(GRP), l(L)] flat [., 128, 2*GRP*L]
    xtg = np.ascontiguousarray(
        emb.transpose(0, 2, 1)                      # [B, D, L]
        .reshape(B // GRP, GRP, 2, 128, L)          # [g, item, d_hi, d_sub, l]
        .transpose(0, 3, 2, 1, 4)                   # [g, d_sub, d_hi, item, l]
        .reshape(B // GRP, 128, 2 * GRP * L)
    ).astype(f8)
    # xl: [B/GRP, l(128), item(GRP), d(D)] flat
    xlg = np.ascontiguousarray(
        emb.reshape(B // GRP, GRP, L, D)
        .transpose(0, 2, 1, 3)
        .reshape(B // GRP, 128, GRP * D)
    ).astype(vdt)
    # rb: [B/GRP, 2(k-tile), item, L]; k-tile 1 = zeros
    rbg = np.zeros((B // GRP, 2, GRP, L), np.float32)
    rbg[:, 0] = rowbias.reshape(B // GRP, GRP, L)
    rbg = rbg.reshape(B // GRP, 2, GRP * L).astype(f8)

    in_maps = []
    gpc_ = BPC // GRP
    for c in range(NCORES):
        sl = slice(c * BPC, (c + 1) * BPC)
        gsl = slice(c * gpc_, (c + 1) * gpc_)
        in_maps.append(
            {
                "xt": xtg[gsl],
                "xl": xlg[gsl],
                "rb": np.ascontiguousarray(rbg[gsl])[None],
                "vt": np.ascontiguousarray(vt_full[:, sl]),
                "aw": awq,
            }
        )
    return in_maps


def _make_exec():
    """Build the shard_map'd PJRT executable once (mirrors
    bass2jax.run_bass_via_pjrt, but returns a reusable callable)."""
    import jax
    from jax.sharding import Mesh, PartitionSpec
    from jax.experimental.shard_map import shard_map
    from concourse import bass2jax, mybir as _mybir

    nc = _get_nc()
    bass2jax.install_neuronx_cc_hook()
    partition_name = nc.partition_id_tensor.name if nc.partition_id_tensor else None
    in_names, out_names, out_avals, zero_outs = [], [], [], []
    for alloc in nc.m.functions[0].allocations:
        if not isinstance(alloc, _mybir.MemoryLocationSet):
            continue
        name = alloc.memorylocations[0].name
        if alloc.kind == "ExternalInput":
            if name != partition_name:
                in_names.append(name)
        elif alloc.kind == "ExternalOutput":
            shape = tuple(alloc.tensor_shape)
            dtype = _mybir.dt.np(alloc.dtype)
            out_names.append(name)
            out_avals.append(jax.core.ShapedArray(shape, dtype))
            zero_outs.append(np.zeros(shape, dtype))
    n_params = len(in_names)
    in_names_full = in_names + out_names
    if partition_name is not None:
        in_names_full.append(partition_name)

    def _body(*args):
        operands = list(args)
        if partition_name is not None:
            operands.append(bass2jax.partition_id_tensor())
        outs = bass2jax._bass_exec_p.bind(
            *operands,
            out_avals=tuple(out_avals),
            in_names=tuple(in_names_full),
            out_names=tuple(out_names),
            lowering_input_output_aliases=(),
            sim_require_finite=True,
            sim_require_nnan=True,
            nc=nc,
        )
        return tuple(outs)

    devices = jax.devices()[:NCORES]
    mesh = Mesh(np.asarray(devices), ("core",))
    n_outs = len(out_names)
    sharded = jax.jit(
        shard_map(
            _body,
            mesh=mesh,
            in_specs=(PartitionSpec("core"),) * (n_params + n_outs),
            out_specs=(PartitionSpec("core"),) * n_outs,
            check_rep=False,
        ),
        donate_argnums=tuple(range(n_params, n_params + n_outs)),
        keep_unused=True,
    )

    def run(in_maps, n_iters=1, timings=None):
        import time as _t

        concat_in = [
            np.concatenate([np.asarray(in_maps[c][nm]) for c in range(NCORES)], axis=0)
            for nm in in_names
        ]
        placed = [jax.device_put(a) for a in concat_in]
        zo = [np.concatenate([z] * NCORES, axis=0) for z in zero_outs]
        outs = None
        for _ in range(n_iters):
            zplaced = [jax.device_put(z) for z in zo]
            for p in placed + zplaced:
                p.block_until_ready()
            t0 = _t.perf_counter()
            outs = sharded(*placed, *zplaced)
            for o in outs:
                o.block_until_ready()
            if timings is not None:
                timings.append(_t.perf_counter() - t0)
        res = []
        for c in range(NCORES):
            d = {}
            for i, nm in enumerate(out_names):
                full = np.asarray(outs[i])
                per = full.shape[0] // NCORES
                d[nm] = full[c * per : (c + 1) * per]
            res.append(d)
        return res

    return run


def _get_runner():
    if "run" not in _CACHE:
        _CACHE["run"] = _make_exec()
    return _CACHE["run"]


def kernel(embeddings, padding_mask, q_w, q_b, k_w, k_b, _n_iters=None, _timings=None):
    in_maps = prep_inputs(embeddings, padding_mask, q_w, q_b, k_w, k_b)
    if _n_iters is None:
        res = run_bass_kernel_spmd(_get_nc(), in_maps, list(range(NCORES)))
        results = res.results
    else:
        results = _get_runner()(in_maps, n_iters=_n_iters, timings=_timings)
    out = np.empty((B, D), np.float32)
    for c in range(NCORES):
        # outT [nchunks, d_sub(128), d_hi(2), b(CHUNK)]
        oT = np.asarray(results[c]["outT"], np.float32).reshape(
            BPC // CHUNK, 128, 2, CHUNK
        )
        blk = oT.transpose(0, 3, 2, 1).reshape(BPC, D)  # [b, d_hi*128+d_sub]
        out[c * BPC : (c + 1) * BPC] = blk
    return out


if __name__ == "__main__":
    ref_inputs = {
        "embeddings": np.random.randn(B, L, D).astype(np.float32),
        "padding_mask": np.random.rand(B, L) < 0.3,
        "q_w": np.random.randn(D, D).astype(np.float32) * 0.06,
        "q_b": np.zeros(D, np.float32),
        "k_w": np.random.randn(D, D).astype(np.float32) * 0.06,
        "k_b": np.zeros(D, np.float32),
    }
    out = kernel(**ref_inputs)
    print(out.shape, out.dtype)


# revision 60
# speedup vs baseline: 1.0052x; 1.0052x over previous
"""Bass/Trainium2 kernel for nn_BatchLoreAttentionLayer.

Reference math (per batch item b, X = embeddings[b] in [L=128, D=256]):
    Q = X q_w^T + q_b ; K = X k_w^T + k_b
    S = Q K^T / sqrt(D) ; S[:, padded] = -inf
    attn = softmax_m(S) ; attended = attn X
    out = tanh( (valid^T attended) / cnt )

Algebraic restructure (same as v1):
    S = X A X^T / sqrt(D) + row_const(l) + s(m),  A = q_w^T k_w
    out_b = tanh( w X ),  w[m] = sum_l g[l] E[l,m],
    E = exp(S + rowbias),  g[l] = valid[b,l] / (rowsum[l] * cnt_b)

v2: fp8 score path with DoubleRow matmuls (89.4us cost model vs 120.9 v1;
    measured end-to-end rel err 1.692e-2 against tol 2e-2, deterministic).
    A and X^T ship as fp8e4m3 in [128, 2, .] two-k-tile layout; Yt = A^T Xt and
    S = Yt^T Xt run as fp8 DoubleRow matmuls (256-contraction, 0.5 cyc/row).
    A is pre-scaled by 256 on host; exp() descales via ACT scale=1/256.
    Mask bias enters S as rank-1 DR matmuls (ones=128 x rb=-160 -> -20480);
    exactly one start=True per touched 2KB PSUM bank and one stop=True on the
    bank's last matmul -- a second start re-marks the whole bank pending-zero
    and later accumulates drop earlier writes (real-HW semantics, matches
    CoreSim's lazy zero-region model).
    Values (xl) ship as fp8e3m4 [l, d] (4 mantissa bits, |X| < 15.5); out bf16.

    Engine split per 8-item group (GPSIMD cannot touch PSUM or reduce along
    the free axis): PE all matmuls; ACT exp + 2 Yt-granule copies (the wall,
    ~2.33us/slot, 100% busy in steady state); DVE 2 Yt copies + short rowsum
    reduce + recip + w copy; GPSIMD two fold-adds (rowsum tree) + g-mul;
    DMA 17.3MB/core at 360B/ns. 8 PSUM banks: 3-buf Yt ring (4 granules +
    w tile) | 2x 2-bank S | 1 out. Slot-pipelined emission with per-stage
    lags (s-1, exp-2, fold-3/4, red-5, gmul-6, w-7, o-8); per-granule SBUF
    tiles avoid cross-engine per-tile WAW serialization; all recurring DMA
    issues on the SP queue (an ACT-queue DMA issue stalls exp dispatch);
    one-time loads spread over slots 0/5/8 to stay under the 8 in-flight
    DMA lane limit.

Sharding: pure data-parallel over B across 8 cores (256 items/core).
"""

import os
import sys
from contextlib import ExitStack

import numpy as np
import ml_dtypes

sys.path.insert(0, "/opt/trn_rl_repo")

import concourse.bass as bass  # noqa: E402
import concourse.mybir as mybir  # noqa: E402
import concourse.tile as tile  # noqa: E402
from concourse import bacc  # noqa: E402
from concourse.bass import ts  # noqa: E402
from concourse.bass_utils import run_bass_kernel_spmd  # noqa: E402

B, L, D = 2048, 128, 256
NCORES = 8
BPC = B // NCORES   # items per core
CHUNK = 128         # items per output accumulation chunk
GRP = 8             # items per vector-op group
NG = BPC // GRP     # groups per core
GPC = CHUNK // GRP  # groups per chunk

F32 = mybir.dt.float32
BF16 = mybir.dt.bfloat16
FP8 = mybir.dt.float8e4    # e4m3, DoubleRow capable
FP8V = mybir.dt.float8e3   # e3m4, values
AF = mybir.ActivationFunctionType
DR = mybir.MatmulPerfMode.DoubleRow

# host-tunable knobs (affect compiled program; keep consistent per process)
VAL_BF16 = os.environ.get("KVAL", "e3") == "bf16"  # values dtype
POOL_COPIES = os.environ.get("KPOOL", "1") == "1"  # gpsimd yt copies
ASC = 256.0  # A pre-scale

_CACHE = {}


def build_bass():
    nc = bacc.Bacc(None, target_bir_lowering=False)
    xt = nc.declare_dram_parameter("xt", [NG, 128, 2 * GRP * L], FP8, isOutput=False)
    xl = nc.declare_dram_parameter(
        "xl", [NG, 128, GRP * D], BF16 if VAL_BF16 else FP8V, isOutput=False
    )
    rb = nc.declare_dram_parameter("rb", [1, NG, 2, GRP * L], FP8, isOutput=False)
    vt = nc.declare_dram_parameter("vt", [128, BPC], F32, isOutput=False)
    aw = nc.declare_dram_parameter("aw", [128, 2 * D], FP8, isOutput=False)
    outT = nc.declare_dram_parameter(
        "outT", [BPC // CHUNK, 128, 2, CHUNK], BF16, isOutput=True
    )
    build_body(nc, xt, xl, rb, vt, aw, outT)
    nc.finalize()
    return nc


def build_body(nc, xt, xl, rb, vt, aw, outT):
    with tile.TileContext(nc) as tc, ExitStack() as ctx:
        singles = ctx.enter_context(tc.tile_pool(name="singles", bufs=1))
        io_xt = ctx.enter_context(tc.tile_pool(name="io_xt", bufs=6))
        io_xl = ctx.enter_context(tc.tile_pool(name="io_xl", bufs=6))
        work = ctx.enter_context(tc.tile_pool(name="work", bufs=12))
        pool_e = ctx.enter_context(tc.tile_pool(name="pool_e", bufs=8))
        small = ctx.enter_context(tc.tile_pool(name="small", bufs=6))
        ps_yt = ctx.enter_context(tc.tile_pool(name="ps_yt", bufs=3, space="PSUM"))
        ps_s = ctx.enter_context(tc.tile_pool(name="ps_s", bufs=2, space="PSUM"))
        ps_o = ctx.enter_context(tc.tile_pool(name="ps_o", bufs=1, space="PSUM"))

        # ---- one-time loads (aw first: it gates the first yt matmul; the
        # bulk rb/vt loads go behind the first xt prefetches) ----
        a_sb = singles.tile([128, 2, D], FP8)  # [d_sub, d_hi, e], A*ASC
        nc.scalar.dma_start(out=a_sb, in_=aw.rearrange("p (t e) -> p t e", t=2))
        ones_dr = singles.tile([1, 2, 128], FP8)
        nc.vector.memset(ones_dr, 128.0)
        vt_sb = singles.tile([128, BPC], F32)  # valid/cnt, [l, b]
        rb_sb = singles.tile([1, NG, 2, GRP * L], FP8)

        def load_singles(step):
            # spread across early slots: <=8 DMAs in flight (DMAHW lanes) and
            # the ACT SEQ stays free for the first granule-copy dispatches
            half = NG // 2
            if step == 0:
                nc.scalar.dma_start(out=rb_sb[:, 0:half], in_=rb[:, 0:half])
            elif step == 5:
                nc.scalar.dma_start(out=vt_sb, in_=vt[:, :])
            elif step == 8:
                nc.scalar.dma_start(out=rb_sb[:, half:NG], in_=rb[:, half:NG])

        xt8 = [None] * NG
        xlv = [None] * NG
        yt_sb = [None] * NG
        sps = [None] * NG
        e_sb = [None] * NG
        rinv = [None] * NG
        g8 = [None] * NG
        w8 = [None] * NG
        oT_ps = [None] * (BPC // CHUNK)
        oT_sb = [None] * (BPC // CHUNK)

        def load_xt(g):
            xt8[g] = io_xt.tile([128, 2, GRP * L], FP8, tag="xt", name="xt8")
            nc.sync.dma_start(
                out=xt8[g], in_=xt.rearrange("n p (t x) -> n p t x", t=2)[g]
            )

        def load_xl(g):
            xlv[g] = io_xl.tile(
                [128, GRP * D], BF16 if VAL_BF16 else FP8V, tag="xl", name="xlv"
            )
            nc.sync.dma_start(out=xlv[g], in_=xl[g])

        def yt_mms(g):
            # Yt = (A*ASC)^T Xt in 4 granules of 2 items; copy to SBUF fp8.
            # Each granule gets its own SBUF tile so the four copies (on
            # different engines) carry no WAW ordering between them.
            yt_sb[g] = []
            for gr in range(4):
                ytp = ps_yt.tile([128, 2, 2 * L], F32, tag="yt")
                for h in range(2):
                    nc.tensor.matmul(
                        out=ytp[:, h, :],
                        lhsT=a_sb[:, :, ts(h, 128)],
                        rhs=xt8[g][:, :, ts(gr, 2 * L)],
                        start=True,
                        stop=True,
                        perf_mode=DR,
                    )
                dst = work.tile([128, 2, 2 * L], FP8, tag="ytsb", name="ytg")
                yt_sb[g].append(dst)
                if gr < 2:  # GPSIMD cannot read PSUM: copies on ACT + DVE
                    nc.scalar.activation(out=dst, in_=ytp, func=AF.Copy)
                else:
                    nc.vector.tensor_copy(out=dst, in_=ytp)

        def s_mms(g):
            # S*ASC for 8 items into one 2-bank PSUM tile + rank-1 mask bias.
            sp = ps_s.tile([128, GRP, L], F32, tag="s", name="spst")
            sps[g] = sp
            sp_flat = sp.rearrange("p g l -> p (g l)")
            # One start per 2KB PSUM bank (a second start re-marks the whole
            # bank pending-zero and the later accumulates drop prior writes);
            # one stop per bank on the last matmul touching it.
            for q in range(4):  # bias first
                nc.tensor.matmul(
                    out=sp_flat[:, ts(q, 2 * L)],
                    lhsT=ones_dr,
                    rhs=rb_sb[:, g, :, ts(q, 2 * L)],
                    start=(q % 2 == 0),
                    stop=False,
                    perf_mode=DR,
                )
            for j in range(GRP):
                nc.tensor.matmul(
                    out=sp[:, j, :],
                    lhsT=yt_sb[g][j // 2][:, :, ts(j % 2, L)],
                    rhs=xt8[g][:, :, ts(j, L)],
                    start=False,
                    stop=(j % 4 == 3),
                    perf_mode=DR,
                )

        def exp_phase(g):
            e_sb[g] = pool_e.tile([128, GRP, L], BF16, tag="e", name="esb")
            nc.scalar.activation(
                out=e_sb[g], in_=sps[g], func=AF.Exp, scale=float(1.0 / ASC)
            )

        ef1 = [None] * NG
        ef2 = [None] * NG

        def fold1_phase(g):
            # rowsum stage 1 on GPSIMD (SBUF->SBUF): E halves added, f32 out
            ef1[g] = pool_e.tile([128, GRP, L // 2], F32, tag="ef1", name="ef1t")
            nc.gpsimd.tensor_add(
                ef1[g], e_sb[g][:, :, 0 : L // 2], e_sb[g][:, :, L // 2 : L]
            )

        def fold2_phase(g):
            ef2[g] = pool_e.tile([128, GRP, L // 4], F32, tag="ef2", name="ef2t")
            nc.gpsimd.tensor_add(
                ef2[g], ef1[g][:, :, 0 : L // 4], ef1[g][:, :, L // 4 : L // 2]
            )

        def red_phase(g):
            rs = small.tile([128, GRP], F32, tag="rs")
            if g >= NG - 2:  # tail: direct reduce, no fold latency in drain
                nc.vector.reduce_sum(
                    out=rs, in_=e_sb[g], axis=mybir.AxisListType.X
                )
            else:
                nc.vector.reduce_sum(
                    out=rs, in_=ef2[g], axis=mybir.AxisListType.X
                )
            rinv[g] = small.tile([128, GRP], F32, tag="rinv", name="rinvt")
            nc.vector.reciprocal(out=rinv[g], in_=rs)

        def gmul_phase(g):
            g8[g] = small.tile([128, GRP], BF16, tag="g8", name="g8t")
            nc.gpsimd.tensor_mul(
                g8[g], rinv[g], vt_sb[:, g * GRP : (g + 1) * GRP]
            )

        def w_mms(g):
            # w[m] = sum_l E[l,m] g[l] per item -> PSUM, copy to SBUF bf16.
            wps = ps_yt.tile([128, GRP], F32, tag="yt", name="wpst")
            for j in range(GRP):
                nc.tensor.matmul(
                    out=wps[:, j : j + 1],
                    lhsT=e_sb[g][:, j, :],
                    rhs=g8[g][:, j : j + 1],
                    start=True,
                    stop=True,
                )
            w8[g] = small.tile([128, GRP], BF16, tag="w8", name="w8t")
            nc.vector.tensor_copy(out=w8[g], in_=wps)

        def o_mms(g):
            c = g // GPC
            if g % GPC == 0:
                oT_ps[c] = ps_o.tile([128, 2, CHUNK], F32, tag="oT", name="oTps")
            col0 = (g % GPC) * GRP
            for j in range(GRP):
                for dh in range(2):
                    nc.tensor.matmul(
                        out=oT_ps[c][:, dh, col0 + j : col0 + j + 1],
                        lhsT=xlv[g].rearrange("p (i d) -> p i d", i=GRP)[
                            :, j, ts(dh, 128)
                        ],
                        rhs=w8[g][:, j : j + 1],
                        start=True,
                        stop=True,
                    )
            if g % GPC == GPC - 1:  # chunk complete -> tanh + store
                oT_sb[c] = work.tile(
                    [128, 2, CHUNK], BF16, tag="oT_sb", name="oTsb"
                )
                nc.scalar.activation(out=oT_sb[c], in_=oT_ps[c], func=AF.Tanh)
                nc.sync.dma_start(out=outT[c], in_=oT_sb[c])

        # ---- flat slot pipeline: each engine's stream lags the chain so no
        # in-order queue waits on a dependency that isn't already satisfied.
        PF = 3
        for slot in range(NG + 9):
            if slot == 0:
                for i in range(PF + 1):
                    load_xt(i)
            if slot in (0, 5, 8):
                load_singles(slot)
            if 2 <= slot and slot - 2 < NG:
                exp_phase(slot - 2)
            if 1 <= slot and slot - 1 < NG:
                s_mms(slot - 1)
            if slot < NG:
                yt_mms(slot)
            if 3 <= slot and slot - 3 < NG - 2:
                fold1_phase(slot - 3)
            if 4 <= slot and slot - 4 < NG - 2:
                fold2_phase(slot - 4)
            if 3 <= slot and NG - 2 <= slot - 3 < NG:
                red_phase(slot - 3)
            if 5 <= slot and slot - 5 < NG - 2:
                red_phase(slot - 5)
            if 4 <= slot and NG - 2 <= slot - 4 < NG:
                gmul_phase(slot - 4)
            if 6 <= slot and slot - 6 < NG - 2:
                gmul_phase(slot - 6)
            if 7 <= slot and slot - 7 < NG:
                w_mms(slot - 7)
            if 8 <= slot and slot - 8 < NG:
                o_mms(slot - 8)
            if 1 <= slot and slot + PF < NG:
                load_xt(slot + PF)
            if 3 <= slot and slot - 3 < NG:
                load_xl(slot - 3)


def _get_nc():
    if "nc" not in _CACHE:
        _CACHE["nc"] = build_bass()
    return _CACHE["nc"]


def prep_inputs(embeddings, padding_mask, q_w, q_b, k_w, k_b):
    """Host-side shard prep: dtype casts, weight folding, mask/count folding."""
    emb = np.asarray(embeddings, np.float32)
    mask = np.asarray(padding_mask)
    q_w = np.asarray(q_w, np.float32)
    k_w = np.asarray(k_w, np.float32)
    q_b = np.asarray(q_b, np.float32)
    scale = 1.0 / np.sqrt(np.float32(D))

    f8 = ml_dtypes.float8_e4m3
    bf = ml_dtypes.bfloat16
    vdt = bf if VAL_BF16 else ml_dtypes.float8_e3m4

    A = (q_w.T @ k_w) * (scale * ASC)              # [D, D] (d, e), pre-scaled
    v = (k_w.T @ q_b) * scale                      # [D]
    # bias rows enter S*ASC via ones(=128) x rb -> 128*rb; exp descales by ASC
    rowbias = np.where(mask, np.float32(-160.0), np.float32(0.0))
    if np.any(v):
        rowbias = rowbias + (emb @ v) * np.float32(ASC / 128.0)
    valid = (~mask).astype(np.float32)             # [B, L]
    cnt = np.maximum(valid.sum(1, keepdims=True), 1.0)
    vt_full = (valid / cnt).T.astype(np.float32)   # [L, B]

    # aw: [d_sub(128), d_hi(2), e(256)] flattened to [128, 512]
    awq = np.ascontiguousarray(
        A.reshape(2, 128, D).transpose(1, 0, 2).reshape(128, 2 * D)
    ).astype(f8)
    # xt: [B/GRP, d_sub(128), d_hi(2), item(GRP), l(L)] flat [., 128, 2*GRP*L]
    xtg = np.ascontiguousarray(
        emb.transpose(0, 2, 1)                      # [B, D, L]
        .reshape(B // GRP, GRP, 2, 128, L)          # [g, item, d_hi, d_sub, l]
        .transpose(0, 3, 2, 1, 4)                   # [g, d_sub, d_hi, item, l]
        .reshape(B // GRP, 128, 2 * GRP * L)
    ).astype(f8)
    # xl: [B/GRP, l(128), item(GRP), d(D)] flat
    xlg = np.ascontiguousarray(
        emb.reshape(B // GRP, GRP, L, D)
        .transpose(0, 2, 1, 3)
        .reshape(B // GRP, 128, GRP * D)
    ).astype(vdt)
    # rb: [B/GRP, 2(k-tile), item, L]; k-tile 1 = zeros
    rbg = np.zeros((B // GRP, 2, GRP, L), np.float32)
    rbg[:, 0] = rowbias.reshape(B // GRP, GRP, L)
    rbg = rbg.reshape(B // GRP, 2, GRP * L).astype(f8)

    in_maps = []
    gpc_ = BPC // GRP
    for c in range(NCORES):
        sl = slice(c * BPC, (c + 1) * BPC)
        gsl = slice(c * gpc_, (c + 1) * gpc_)
        in_maps.append(
            {
                "xt": xtg[gsl],
                "xl": xlg[gsl],
                "rb": np.ascontiguousarray(rbg[gsl])[None],
                "vt": np.ascontiguousarray(vt_full[:, sl]),
                "aw": awq,
            }
        )
    return in_maps


def _make_exec():
    """Build the shard_map'd PJRT executable once (mirrors
    bass2jax.run_bass_via_pjrt, but returns a reusable callable)."""
    import jax
    from jax.sharding import Mesh, PartitionSpec
    from jax.experimental.shard_map import shard_map
    from concourse import bass2jax, mybir as _mybir

    nc = _get_nc()
    bass2jax.install_neuronx_cc_hook()
    partition_name = nc.partition_id_tensor.name if nc.partition_id_tensor else None
    in_names, out_names, out_avals, zero_outs = [], [], [], []
    for alloc in nc.m.functions[0].allocations:
        if not isinstance(alloc, _mybir.MemoryLocationSet):
            continue
        name = alloc.memorylocations[0].name
        if alloc.kind == "ExternalInput":
            if name != partition_name:
                in_names.append(name)
        elif alloc.kind == "ExternalOutput":
            shape = tuple(alloc.tensor_shape)
            dtype = _mybir.dt.np(alloc.dtype)
            out_names.append(name)
            out_avals.append(jax.core.ShapedArray(shape, dtype))
            zero_outs.append(np.zeros(shape, dtype))
    n_params = len(in_names)
    in_names_full = in_names + out_names
    if partition_name is not None:
        in_names_full.append(partition_name)

    def _body(*args):
        operands = list(args)
        if partition_name is not None:
            operands.append(bass2jax.partition_id_tensor())
        outs = bass2jax._bass_exec_p.bind(
            *operands,
            out_avals=tuple(out_avals),
            in_names=tuple(in_names_full),
            out_names=tuple(out_names),
            lowering_input_output_aliases=(),
            sim_require_finite=True,
            sim_require_nnan=True,
            nc=nc,
        )
        return tuple(outs)

    devices = jax.devices()[:NCORES]
    mesh = Mesh(np.asarray(devices), ("core",))
    n_outs = len(out_names)
    sharded = jax.jit(
        shard_map(
            _body,
            mesh=mesh,
            in_specs=(PartitionSpec("core"),) * (n_params + n_outs),
            out_specs=(PartitionSpec("core"),) * n_outs,
            check_rep=False,
        ),
        donate_argnums=tuple(range(n_params, n_params + n_outs)),
        keep_unused=True,
    )

    def run(in_maps, n_iters=1, timings=None):
        import time as _t

        concat_in = [
            np.concatenate([np.asarray(in_maps[c][nm]) for c in range(NCORES)], axis=0)
            for nm in in_names
        ]
        placed = [jax.device_put(a) for a in concat_in]
        zo = [np.concatenate([z] * NCORES, axis=0) for z in zero_outs]
        outs = None
        for _ in range(n_iters):
            zplaced = [jax.device_put(z) for z in zo]
            for p in placed + zplaced:
                p.block_until_ready()
            t0 = _t.perf_counter()
            outs = sharded(*placed, *zplaced)
            for o in outs:
                o.block_until_ready()
            if timings is not None:
                timings.append(_t.perf_counter() - t0)
        res = []
        for c in range(NCORES):
            d = {}
            for i, nm in enumerate(out_names):
                full = np.asarray(outs[i])
                per = full.shape[0] // NCORES
                d[nm] = full[c * per : (c + 1) * per]
            res.append(d)
        return res

    return run


def _get_runner():
    if "run" not in _CACHE:
        _CACHE["run"] = _make_exec()
    return _CACHE["run"]


def kernel(embeddings, padding_mask, q_w, q_b, k_w, k_b, _n_iters=None, _timings=None):
    in_maps = prep_inputs(embeddings, padding_mask, q_w, q_b, k_w, k_b)
    if _n_iters is None:
        res = run_bass_kernel_spmd(_get_nc(), in_maps, list(range(NCORES)))
        results = res.results
    else:
        results = _get_runner()(in_maps, n_iters=_n_iters, timings=_timings)
    out = np.empty((B, D), np.float32)
    for c in range(NCORES):
        # outT [nchunks, d_sub(128), d_hi(2), b(CHUNK)]
        oT = np.asarray(results[c]["outT"], np.float32).reshape(
            BPC // CHUNK, 128, 2, CHUNK
        )
        blk = oT.transpose(0, 3, 2, 1).reshape(BPC, D)  # [b, d_hi*128+d_sub]
        out[c * BPC : (c + 1) * BPC] = blk
    return out


if __name__ == "__main__":
    ref_inputs = {
        "embeddings": np.random.randn(B, L, D).astype(np.float32),
        "padding_mask": np.random.rand(B, L) < 0.3,
        "q_w": np.random.randn(D, D).astype(np.float32) * 0.06,
        "q_b": np.zeros(D, np.float32),
        "k_w": np.random.randn(D, D).astype(np.float32) * 0.06,
        "k_b": np.zeros(D, np.float32),
    }
    out = kernel(**ref_inputs)
    print(out.shape, out.dtype)
